# revision 1
# baseline (speedup 1.0000x reference)
"""Trainium2 Bass kernel for nn_Deep_OSTTP_Model (deep tanh-LN recurrence with decayed trace).

Self-contained: takes FULL inputs, shards batch across 8 NeuronCores (pure data
parallel), runs a Bass/Tile kernel per core, gathers the full output.

Fast path exploits verified input structure:
  - Wz_w[i] == c_i * I  -> the recurrent matmul is a scalar multiply folded into
    LayerNorm algebra: LN(c*h + a) == (v - mean(v)) * rsqrt(var(v) + eps/c^2)
    with v = h + a/c (a = z @ Wx^T, pre-scaled by 1/c on the host).
  - all biases zero, ln_g == 1, ln_b == 0, R finite (the R projection is
    multiplied by 0.0 in the reference).
If any structural check fails, a general numpy fallback computes the exact
reference math on the host.
"""
import numpy as np

L = 4
B = 4096
IN = 2048
H = 1024
OUT = 256
NSTEPS = 8
DECAY = 0.9
LN_EPS = 1e-5

NCORES = 8
BC = B // NCORES          # 512 rows per core
NCH = BC // 128           # 4 chunks of 128 rows
KH = H // 128             # 8 contraction blocks over H
KIN = IN // 128           # 16 contraction blocks over IN
MAGIC = 0x5F3759DF

_cache = {}


def _build_program(c_per_layer, tiny_engine="vector"):
    from contextlib import ExitStack
    import concourse.tile as tile
    from concourse import bacc, mybir

    F32 = mybir.dt.float32
    F32R = mybir.dt.float32r
    U32 = mybir.dt.uint32
    I32 = mybir.dt.int32
    A = mybir.AluOpType
    TANH = mybir.ActivationFunctionType.Tanh

    nc = bacc.Bacc("TRN2", target_bir_lowering=False, debug=False)

    xT_d = nc.dram_tensor("xT", [IN, BC], F32R, kind="ExternalInput").ap()
    pwT_d = nc.dram_tensor("pwT", [IN, H], F32R, kind="ExternalInput").ap()
    wxT_d = nc.dram_tensor("wxT", [L * H, H], F32R, kind="ExternalInput").ap()
    hd_d = nc.dram_tensor("hd", [128, KH * OUT], F32R, kind="ExternalInput").ap()
    idR_d = nc.dram_tensor("idR", [128, NSTEPS * 128], F32R, kind="ExternalInput").ap()
    out_d = nc.dram_tensor("out", [BC, OUT], F32, kind="ExternalOutput").ap()

    with tile.TileContext(nc) as tc, ExitStack() as ctx:
        consts = ctx.enter_context(tc.tile_pool(name="consts", bufs=1))
        idR = consts.tile([128, NSTEPS * 128], F32R)
        nc.sync.dma_start(idR[:], idR_d)
        idP = idR[:, (NSTEPS - 1) * 128: NSTEPS * 128]  # plain identity (w=1)
        hd_sb = consts.tile([128, KH * OUT], F32R)
        nc.sync.dma_start(hd_sb[:], hd_d)

        wxp = ctx.enter_context(tc.tile_pool(name="wxp", bufs=12))
        z0p = ctx.enter_context(tc.tile_pool(name="z0p", bufs=KH))
        # single PSUM pool: 4 x [128,1024] fp32 = all 8 banks
        ps = ctx.enter_context(tc.tile_pool(name="ps", bufs=4, space="PSUM"))

        def pstile(name):
            return ps.tile([128, H], F32, name=name, tag="v", bufs=4)

        # ---------------- Stage A: z0T[h] = sum_k pwT[k][:,h].T @ xT[k] ------
        # 8 H-block accumulators packed 2-per-psum-slot.
        z0T = []
        with tc.tile_pool(name="xA", bufs=4) as xA, \
             tc.tile_pool(name="pwA", bufs=4) as pwA:
            zps = [pstile(f"zpsA{j}") for j in range(KH // 2)]
            for k in range(KIN):
                xk = xA.tile([128, BC], F32R, name=f"xk{k}", tag="xk")
                nc.sync.dma_start(xk[:], xT_d[k * 128:(k + 1) * 128, :])
                pwk = pwA.tile([128, H], F32R, name=f"pwk{k}", tag="pwk")
                nc.scalar.dma_start(pwk[:], pwT_d[k * 128:(k + 1) * 128, :])
                for h in range(KH):
                    dst = zps[h // 2][:, (h % 2) * 512:(h % 2) * 512 + BC]
                    nc.tensor.matmul(dst, pwk[:, h * 128:(h + 1) * 128], xk[:],
                                     start=(k == 0), stop=(k == KIN - 1))
            for h in range(KH):
                z0T_h = z0p.tile([128, BC], F32R, name=f"z0T{h}", tag="z0T")
                src = zps[h // 2][:, (h % 2) * 512:(h % 2) * 512 + BC]
                if h % 2 == 0:
                    nc.scalar.copy(z0T_h[:], src)
                else:
                    nc.vector.tensor_copy(z0T_h[:], src)
                z0T.append(z0T_h)

        # ---------------- steady-state sbuf pools -----------------------------
        zwp = ctx.enter_context(tc.tile_pool(name="zwp", bufs=4))
        hp = ctx.enter_context(tc.tile_pool(name="hp", bufs=18))
        trp = ctx.enter_context(tc.tile_pool(name="trp", bufs=4))
        trTp = ctx.enter_context(tc.tile_pool(name="trTp", bufs=2))
        stp = ctx.enter_context(tc.tile_pool(name="stp", bufs=2))
        outp = ctx.enter_context(tc.tile_pool(name="outp", bufs=2))

        wx_sb = {}

        def load_wx(i):
            tiles = []
            for k in range(KH):
                t = wxp.tile([128, H], F32R, name=f"wx{i}_{k}", tag="wx", bufs=12)
                nc.sync.dma_start(t[:], wxT_d[i * H + k * 128: i * H + (k + 1) * 128, :])
                tiles.append(t)
            wx_sb[i] = tiles

        load_wx(0)

        tiny = nc.gpsimd if tiny_engine == "gpsimd" else nc.vector

        tr_sb = [None] * NCH

        def transposes(c, src, note=""):
            """trT[c]: [128, KH*128] f32r, block k = (src block k).T"""
            trT = trTp.tile([128, KH * 128], F32R, name=f"trT{note}_{c}", tag="trT")
            tp = pstile(f"tps{note}_{c}")
            tpr = tp.bitcast(F32R)
            for k in range(KH):
                nc.tensor.transpose(tpr[:, k * 128:(k + 1) * 128],
                                    src[:, k * 128:(k + 1) * 128], idP)
            if c % 4 == 3:
                nc.vector.tensor_copy(trT[:], tpr[:])
            else:
                nc.scalar.copy(trT[:], tpr[:])
            return trT

        def emit_boundary(i, c):
            """zw_sb[c] for layer i (= z/c @ WxT')."""
            if i == 0:
                def lhsT_k(k, c=c):
                    return z0T[k][:, c * 128:(c + 1) * 128]
            else:
                trT = transposes(c, tr_sb[c], note=f"L{i}")

                def lhsT_k(k, trT=trT):
                    return trT[:, k * 128:(k + 1) * 128]
            zw_ps = pstile(f"zwps{i}_{c}")
            for half in range(2):
                o = zw_ps[:, half * 512:(half + 1) * 512]
                for k in range(KH):
                    nc.tensor.matmul(o, lhsT_k(k),
                                     wx_sb[i][k][:, half * 512:(half + 1) * 512],
                                     start=(k == 0), stop=(k == KH - 1))
            zw_c = zwp.tile([128, H], F32R, name=f"zw{i}_{c}", tag="zw", bufs=4)
            if c % 4 == 3:
                nc.vector.tensor_copy(zw_c[:], zw_ps[:])
            else:
                nc.scalar.copy(zw_c[:], zw_ps[:])
            return zw_c

        def emit_flush(i, c, t_lo, t_hi, hs, first):
            """trace partial: sum_{t=t_lo..t_hi} DECAY^(NSTEPS-1-t) * h_t."""
            fp = pstile(f"fl{i}_{c}_{t_lo}")
            for half in range(2):
                o = fp[:, half * 512:(half + 1) * 512]
                for t in range(t_lo, t_hi + 1):
                    nc.tensor.matmul(o, idR[:, t * 128:(t + 1) * 128],
                                     hs[t][:, half * 512:(half + 1) * 512],
                                     start=(t == t_lo), stop=(t == t_hi))
            if first:
                tr_c = trp.tile([128, H], F32R, name=f"tr{i}_{c}", tag="tr_sb")
                if c % 4 == 3:
                    nc.vector.tensor_copy(tr_c[:], fp[:])
                else:
                    nc.scalar.copy(tr_c[:], fp[:])
                tr_sb[c] = tr_c
            else:
                nc.vector.tensor_tensor(tr_sb[c][:], tr_sb[c][:].bitcast(F32),
                                        fp[:], op=A.add)

        FLUSH_AT = (3, NSTEPS - 1)

        def emit_layer(i, eps_i):
            zw = [emit_boundary(i, c) for c in range(NCH)]
            hs = [dict() for _ in range(NCH)]   # per chunk: t -> h tile
            h_prev = [None] * NCH
            for t in range(NSTEPS):
                v_t = [None] * NCH
                if t > 0:
                    for c in range(NCH):
                        v_ps = pstile(f"v{i}_{c}_{t}")
                        for half in range(2):
                            o = v_ps[:, half * 512:(half + 1) * 512]
                            nc.tensor.matmul(o, idP,
                                             zw[c][:, half * 512:(half + 1) * 512],
                                             start=True, stop=False)
                            nc.tensor.matmul(o, idP,
                                             h_prev[c][:, half * 512:(half + 1) * 512],
                                             start=False, stop=True)
                        v_t[c] = v_ps[:]
                else:
                    for c in range(NCH):
                        v_t[c] = zw[c][:].bitcast(F32)
                st = stp.tile([128, NCH * 12], F32, name=f"st{i}_{t}", tag="st")
                mv = stp.tile([128, NCH * 2], F32, name=f"mv{i}_{t}", tag="mv")
                y4 = stp.tile([128, NCH], F32, name=f"y{i}_{t}", tag="y")
                b4 = stp.tile([128, NCH], F32, name=f"b{i}_{t}", tag="b4")
                nq4 = stp.tile([128, NCH], F32, name=f"nq{i}_{t}", tag="nq")
                t4 = stp.tile([128, NCH], F32, name=f"t{i}_{t}", tag="t4")
                for pr in range(2):
                    cls, chs = 2 * pr, (2 * pr, 2 * pr + 1)
                    for c in chs:
                        for half in range(2):
                            nc.vector.bn_stats(
                                st[:, c * 12 + half * 6: c * 12 + half * 6 + 6],
                                v_t[c][:, half * 512:(half + 1) * 512])
                    for c in chs:
                        nc.vector.bn_aggr(mv[:, c * 2:(c + 1) * 2],
                                          st[:, c * 12:(c + 1) * 12]
                                          .rearrange("p (g s) -> p g s", s=6))
                    mvw = mv[:, cls * 2:(cls + 2) * 2].rearrange("p (c s) -> p s c", s=2)
                    mean2, var2 = mvw[:, 0, :], mvw[:, 1, :]
                    nq2 = nq4[:, cls:cls + 2]
                    y2 = y4[:, cls:cls + 2]
                    t2 = t4[:, cls:cls + 2]
                    b2 = b4[:, cls:cls + 2]
                    # nq = -(var+eps)/2 ; seed y=magic(var) ; 2 Newton iters
                    tiny.tensor_scalar(nq2, var2, eps_i, -0.5, op0=A.add, op1=A.mult)
                    tiny.tensor_scalar(y2.bitcast(U32), var2.bitcast(U32), 1, None,
                                       op0=A.logical_shift_right)
                    tiny.tensor_scalar(y2.bitcast(I32), y2.bitcast(I32), -1,
                                       MAGIC, op0=A.mult, op1=A.add)
                    for _ in range(2):
                        tiny.tensor_tensor(t2, y2, y2, op=A.mult)
                        tiny.tensor_tensor(t2, t2, nq2, op=A.mult)
                        tiny.scalar_tensor_tensor(y2, t2, 1.5, y2,
                                                  op0=A.add, op1=A.mult)
                    tiny.scalar_tensor_tensor(b2, mean2, -1.0, y2,
                                              op0=A.mult, op1=A.mult)
                    for c in chs:
                        h_new = hp.tile([128, H], F32R, name=f"h{i}_{c}_{t}", tag="h")
                        nc.scalar.activation(h_new[:], v_t[c], TANH,
                                             bias=b4[:, c:c + 1], scale=y4[:, c:c + 1])
                        hs[c][t] = h_new
                        h_prev[c] = h_new

                if t in FLUSH_AT:
                    lo = 0 if t == FLUSH_AT[0] else FLUSH_AT[0] + 1
                    for c in range(NCH):
                        emit_flush(i, c, lo, t, hs[c], first=(t == FLUSH_AT[0]))

        for i in range(L):
            eps_i = LN_EPS / (c_per_layer[i] ** 2)
            if 1 <= i < L - 1:
                load_wx(i + 1)
            elif i == 0:
                load_wx(1)
            emit_layer(i, eps_i)

        # ---------------- head ------------------------------------------------
        for c in range(NCH):
            trT = transposes(c, tr_sb[c], note="HD")
            hd_ps = pstile(f"hdps{c}")
            for k in range(KH):
                nc.tensor.matmul(hd_ps[:, :OUT], trT[:, k * 128:(k + 1) * 128],
                                 hd_sb[:, k * OUT:(k + 1) * OUT],
                                 start=(k == 0), stop=(k == KH - 1))
            o_sb = outp.tile([128, OUT], F32, name=f"osb{c}", tag="o")
            if c % 2 == 0:
                nc.scalar.copy(o_sb[:], hd_ps[:, :OUT])
            else:
                nc.vector.tensor_copy(o_sb[:], hd_ps[:, :OUT])
            nc.sync.dma_start(out_d[c * 128:(c + 1) * 128, :], o_sb[:])

    nc.compile()
    return nc


def _fallback_numpy(x, proj_in_w, proj_in_b, Wz_w, Wz_b, Wx_w, ln_g, ln_b, R,
                    head_w, head_b):
    x = x.astype(np.float32)
    z = x @ proj_in_w.T + proj_in_b
    for i in range(L):
        zWx = z @ Wx_w[i].T
        h = np.zeros_like(z)
        hs = []
        for _ in range(NSTEPS):
            u = h @ Wz_w[i].T + Wz_b[i] + zWx
            m = u.mean(axis=-1, keepdims=True)
            var = np.square(u - m).mean(axis=-1, keepdims=True)
            h = np.tanh((u - m) / np.sqrt(var + LN_EPS) * ln_g[i] + ln_b[i])
            hs.append(h)
        tr = np.zeros_like(z)
        for hh in hs:
            tr = DECAY * tr + hh
        dummy = np.ones((x.shape[0], OUT), dtype=x.dtype)
        proj = dummy @ R[i]
        z = tr + proj * 0.0
    return (z @ head_w.T + head_b).astype(np.float32)


def _check_structure(proj_in_b, Wz_w, Wz_b, ln_g, ln_b, R, head_b):
    cs = []
    eye = np.eye(H, dtype=np.float32)
    for i in range(L):
        c = float(Wz_w[i, 0, 0])
        if c <= 0 or not np.array_equal(Wz_w[i], c * eye):
            return None
        cs.append(c)
    if not (np.all(Wz_b == 0) and np.all(proj_in_b == 0) and np.all(head_b == 0)
            and np.all(ln_g == 1) and np.all(ln_b == 0) and np.all(np.isfinite(R))):
        return None
    return tuple(cs)


def _prep_in_maps(np_in, cs):
    x = np_in["x"].astype(np.float32, copy=False)
    pwT = np.ascontiguousarray(np_in["proj_in_w"].astype(np.float32).T)
    wxT = np.concatenate(
        [np.ascontiguousarray(np_in["Wx_w"][i].astype(np.float32).T) / cs[i]
         for i in range(L)], axis=0)
    hd = np.ascontiguousarray(
        np_in["head_w"].astype(np.float32).T.reshape(KH, 128, OUT)
        .transpose(1, 0, 2).reshape(128, KH * OUT))
    idR = np.zeros((128, NSTEPS * 128), dtype=np.float32)
    for t in range(NSTEPS):
        idR[:, t * 128:(t + 1) * 128] = (DECAY ** (NSTEPS - 1 - t)) * np.eye(
            128, dtype=np.float32)
    in_maps = []
    for s in range(NCORES):
        xT = np.ascontiguousarray(x[s * BC:(s + 1) * BC, :].T)
        in_maps.append({"xT": xT, "pwT": pwT, "wxT": wxT, "hd": hd, "idR": idR})
    return in_maps


def kernel(**inputs):
    np_in = {k: np.asarray(v) for k, v in inputs.items()}
    cs = _check_structure(np_in["proj_in_b"], np_in["Wz_w"], np_in["Wz_b"],
                          np_in["ln_g"], np_in["ln_b"], np_in["R"], np_in["head_b"])
    if cs is None:
        return _fallback_numpy(**np_in)

    import concourse.bass_utils as bass_utils

    if cs not in _cache:
        _cache[cs] = _build_program(cs)
    nc = _cache[cs]
    in_maps = _prep_in_maps(np_in, cs)
    res = bass_utils.run_bass_kernel_spmd(nc, in_maps, core_ids=list(range(NCORES)))
    out = np.concatenate([res.results[s]["out"] for s in range(NCORES)], axis=0)
    return out.astype(np.float32)


def run_traced(np_in, trace_cores=None):
    """Run with NTFF tracing enabled; returns BassKernelResults."""
    import concourse.bass_utils as bass_utils
    np_in = {k: np.asarray(v) for k, v in np_in.items()}
    cs = _check_structure(np_in["proj_in_b"], np_in["Wz_w"], np_in["Wz_b"],
                          np_in["ln_g"], np_in["ln_b"], np_in["R"], np_in["head_b"])
    assert cs is not None
    if cs not in _cache:
        _cache[cs] = _build_program(cs)
    nc = _cache[cs]
    in_maps = _prep_in_maps(np_in, cs)
    return bass_utils.run_bass_kernel_spmd(
        nc, in_maps, core_ids=list(range(NCORES)), trace=True,
        trace_cores=trace_cores)

